# revision 15
# baseline (speedup 1.0000x reference)
import sys

sys.path.insert(0, "/opt/trn_rl_repo")
import numpy as np

N, E, F, L, R = 40000, 400000, 128, 3, 510
CUTOFF, GAP = 51.0, 0.1
NCORES = 8
NPN = 5000          # real nodes per core
NPC = 5120          # padded nodes per core (40 groups x 128)
NG = NPC // 128     # 40 node groups per core
HALF = 32768        # int16 gather lo/hi table split
RANK = 64           # dist-basis rank
RB = RANK + 1       # + bias column
GRIDM = 4096
ECHUNK = 1024       # edges per gather/compute chunk (8 subtiles)

f16d = np.float16
f32d = np.float32


def _sp(x):
    return np.where(0.5 * x > 14.0, x, 2.0 * np.log1p(np.exp(np.minimum(0.5 * x, 30.0))))


def _wrap16(idx):
    # gather idx layout: idx j at (partition 16g + j%16, col j//16); uploaded
    # as the compact 16-partition block, replicated on-device across the
    # eight 16-partition groups (each GPSIMD cpu reads its own slice)
    return np.ascontiguousarray(idx.reshape(-1, 16).T.astype(np.int16))


def _host_prep(inp):
    nt = np.asarray(inp["nfeats"])[:, 0].astype(np.int64)
    src = np.asarray(inp["src"]).astype(np.int64)
    dst = np.asarray(inp["dst"]).astype(np.int64)
    ef = np.asarray(inp["efeats"]).astype(f32d)
    dist = np.linalg.norm(ef, axis=1)

    centers = np.linspace(0.0, CUTOFF, R).astype(f32d)
    glo, ghi = dist.min() - 0.01, dist.max() + 0.01
    grid = np.linspace(glo, ghi, GRIDM)
    rbf_g = np.exp(-(1.0 / GAP) * (grid[:, None] - centers[None, :]) ** 2)
    G = np.hstack([_sp(rbf_g @ inp["d1_W"][l] + inp["d1_b"][l]) for l in range(L)])
    U, S, Vt = np.linalg.svd(G, full_matrices=False)
    Phi = U[:, :RANK] * np.sqrt(S[:RANK])
    C = np.sqrt(S[:RANK])[:, None] * Vt[:RANK]

    Up = np.empty((E, RB), f32d)
    step = (ghi - glo) / (GRIDM - 1)
    pos = (dist - glo) / step
    kk = np.clip(pos.astype(np.int64), 0, GRIDM - 2)
    frac = (pos - kk)[:, None]
    Up[:, :RANK] = Phi[kk] * (1.0 - frac) + Phi[kk + 1] * frac
    Up[:, RANK] = 1.0
    Ceff = np.stack(
        [
            np.vstack([C[:, 128 * l : 128 * l + 128] @ inp["d2_W"][l], inp["d2_b"][l][None]])
            for l in range(L)
        ]
    ).astype(f16d)  # [L, RB, 128]

    # e path: e has <=3 distinct rows indexed by etype in {0,1,3}
    emap = np.zeros(4, np.int64)
    emap[[0, 1, 3]] = [0, 1, 2]
    etype = emap[nt[src] * nt[dst] + nt[src] + nt[dst]]
    e_cur = np.asarray(inp["edge_emb"])[[0, 1, 3]].astype(f32d)
    e2s = []
    for l in range(L):
        e2 = e_cur @ inp["eu_W"][l] + inp["eu_b"][l]
        e2s.append(e2.astype(f16d))
        e_cur = _sp(e2 @ inp["el1_W"][l] + inp["el1_b"][l])
    e2w = np.stack(e2s)  # [L, 3, 128]

    cnt = np.zeros((N, 3), f32d)
    np.add.at(cnt, (dst, etype), 1.0)

    # node remap: node n -> row 5120*(n//5000) + n%5000
    newsrc = NPC * (src // NPN) + src % NPN

    # per-(group,half) padded segment sizes, shared across cores (SPMD)
    core = dst // NPN
    ld = dst - NPN * core
    gq = ld // 128
    loc = ld % 128
    hf = (newsrc >= HALF).astype(np.int64)
    segc = np.zeros((NCORES, NG, 2), np.int64)
    np.add.at(segc, (core, gq, hf), 1)
    P = 128 * np.ceil(segc.max(axis=0) / 128).astype(np.int64)  # [NG, 2]
    Llo, Lhi = int(P[:, 0].sum()), int(P[:, 1].sum())
    EP = Llo + Lhi

    Upt16 = Up.astype(f16d)
    percore = []
    for k in range(NCORES):
        m = np.where(core == k)[0]
        gsrc = np.zeros(EP, np.int64)
        dloc = np.full(EP, 999.0, f32d)
        upc = np.zeros((EP, RB), f16d)
        pos = 0
        for h in (0, 1):
            for g in range(NG):
                sel = m[(hf[m] == h) & (gq[m] == g)]
                n = len(sel)
                gsrc[pos : pos + n] = newsrc[sel] - (HALF if h else 0)
                dloc[pos : pos + n] = loc[sel]
                upc[pos : pos + n] = Upt16[sel]
                pos += int(P[g, h])
        assert pos == EP
        ES = EP // 128
        dstem = np.zeros((128, ES), f16d)
        dstem[:, :] = dloc.reshape(ES, 128).T  # edge i -> [i%128, i//128]
        nloc = np.arange(NPC)
        glob = NPN * k + np.minimum(nloc, NPN - 1)
        h0 = np.asarray(inp["node_emb"])[nt[glob]].astype(f32d)
        h0[nloc >= NPN] = 0.0
        percore.append(
            dict(
                gidx_lo=_wrap16(gsrc[:Llo]),
                gidx_hi=_wrap16(gsrc[Llo:]),
                dstem=dstem,
                UpT=np.ascontiguousarray(upc.T),  # [RB, EP]
                cntT=np.ascontiguousarray(cnt[NPN * k : NPN * (k + 1)].T.astype(f16d)),
                h016=np.ascontiguousarray(h0.T.astype(f16d)),
            )
        )
    for pc in percore:
        z = np.zeros((3, NPC), f16d)
        z[:, :NPN] = pc["cntT"]
        pc["cntT"] = z

    wts = dict(
        nl1W=np.ascontiguousarray(np.concatenate([inp["nl1_W"][l] for l in range(L)], axis=1)).astype(f16d),  # [128, 3*128]
        Ceff=np.ascontiguousarray(np.concatenate([Ceff[l] for l in range(L)], axis=1)),  # [RB, 3*128]
        e2w=np.ascontiguousarray(np.concatenate([e2w[l] for l in range(L)], axis=1)),  # [3, 3*128]
        nl2W=np.ascontiguousarray(np.concatenate([inp["nl2_W"][l] for l in range(L)], axis=1)).astype(f16d),
        nl2bh=np.ascontiguousarray(np.stack([0.5 * inp["nl2_b"][l] for l in range(L)], axis=1)).astype(f32d),  # [128, L]
        nl3W2=np.ascontiguousarray(np.concatenate([2.0 * inp["nl3_W"][l] for l in range(L)], axis=1)).astype(f16d),
        dc0W=np.ascontiguousarray(
            np.concatenate([inp["dec0_W"][128 * l : 128 * l + 128] for l in range(4)], axis=1)
        ).astype(f16d),  # [128, 512]
        dc1W=inp["dec1_W"].astype(f16d),
        dc2W=inp["dec2_W"].astype(f16d),
        dc3W=inp["dec3_W"].astype(f16d),
        dc4W=inp["dec4_W"].astype(f16d),  # [128, 1]
        iota=np.tile(np.arange(128, dtype=f16d), (128, 1)),
    )
    prelu_a = [float(a) for a in np.asarray(inp["prelu_a"])]

    # subtile metadata shared across cores
    def submeta(col):
        subs = []
        for g in range(NG):
            n = int(P[g, col]) // 128
            for j in range(n):
                subs.append((g, j == 0, j == n - 1))
        return subs

    layout = dict(
        P=P,
        Llo=Llo,
        Lhi=Lhi,
        EP=EP,
        subs_lo=submeta(0),
        subs_hi=submeta(1),
        empty_lo=[g for g in range(NG) if P[g, 0] == 0],
        prelu_a=prelu_a,
    )
    return percore, wts, layout


def _build(layout):
    from concourse import bacc, tile, mybir

    f16 = mybir.dt.float16
    f32 = mybir.dt.float32
    i16 = mybir.dt.int16
    AF = mybir.ActivationFunctionType
    OP = mybir.AluOpType

    Llo, Lhi, EP = layout["Llo"], layout["Lhi"], layout["EP"]
    ES = EP // 128
    ESlo = Llo // 128
    subs_lo, subs_hi = layout["subs_lo"], layout["subs_hi"]
    prelu_a = layout["prelu_a"]

    nc = bacc.Bacc(
        "TRN2",
        target_bir_lowering=False,
        debug=False,
        enable_asserts=False,
        num_devices=NCORES,
    )

    p = {}
    p["gidx_lo"] = nc.declare_dram_parameter("gidx_lo", [16, Llo // 16], i16, isOutput=False)
    p["gidx_hi"] = nc.declare_dram_parameter("gidx_hi", [16, Lhi // 16], i16, isOutput=False)
    p["dstem"] = nc.declare_dram_parameter("dstem", [128, ES], f16, isOutput=False)
    p["UpT"] = nc.declare_dram_parameter("UpT", [RB, EP], f16, isOutput=False)
    p["cntT"] = nc.declare_dram_parameter("cntT", [3, NPC], f16, isOutput=False)
    p["h016"] = nc.declare_dram_parameter("h016", [128, NPC], f16, isOutput=False)
    p["iota"] = nc.declare_dram_parameter("iota", [128, 128], f16, isOutput=False)
    p["nl1W"] = nc.declare_dram_parameter("nl1W", [128, 3 * 128], f16, isOutput=False)
    p["Ceff"] = nc.declare_dram_parameter("Ceff", [RB, 3 * 128], f16, isOutput=False)
    p["e2w"] = nc.declare_dram_parameter("e2w", [3, 3 * 128], f16, isOutput=False)
    p["nl2W"] = nc.declare_dram_parameter("nl2W", [128, 3 * 128], f16, isOutput=False)
    p["nl2bh"] = nc.declare_dram_parameter("nl2bh", [128, L], f32, isOutput=False)
    p["nl3W2"] = nc.declare_dram_parameter("nl3W2", [128, 3 * 128], f16, isOutput=False)
    p["dc0W"] = nc.declare_dram_parameter("dc0W", [128, 512], f16, isOutput=False)
    for i in (1, 2, 3):
        p[f"dc{i}W"] = nc.declare_dram_parameter(f"dc{i}W", [128, 128], f16, isOutput=False)
    p["dc4W"] = nc.declare_dram_parameter("dc4W", [128, 1], f16, isOutput=False)
    out = nc.declare_dram_parameter("out", [1, NPC], f32, isOutput=True)

    ag_in = [nc.dram_tensor(f"ag_in{l}", [NPC, 128], f16) for l in range(L)]
    hn_all = [
        nc.dram_tensor(f"hn_all{l}", [NCORES * NPC, 128], f16, addr_space="Shared")
        for l in range(L)
    ]
    snapd = [nc.dram_tensor(f"snap{l}", [128, NPC], f16) for l in (1, 2)]

    with tile.TileContext(nc) as tc:
        with (
            tc.tile_pool(name="persist", bufs=1) as pp,
            tc.tile_pool(name="gpool", bufs=2) as gp,
            tc.tile_pool(name="upool", bufs=2) as up,
            tc.tile_pool(name="spool", bufs=8) as sp,
            tc.tile_pool(name="npool", bufs=6) as npo,
            tc.tile_pool(name="psD", bufs=2, space="PSUM") as psD,
            tc.tile_pool(name="psA", bufs=2, space="PSUM") as psA,
            tc.tile_pool(name="psN", bufs=2, space="PSUM") as psN,
        ):
            # persistent loads
            t = {}
            for nm in ("gidx_lo", "gidx_hi"):
                width = (Llo if nm == "gidx_lo" else Lhi) // 16
                t[nm] = pp.tile([128, width], i16, name=f"t_{nm}")
                for g in range(8):
                    nc.sync.dma_start(t[nm][16 * g : 16 * (g + 1), :], p[nm][:])
            for nm, shp, dt in (
                ("dstem", [128, ES], f16),
                ("cntT", [3, NPC], f16),
                ("iota", [128, 128], f16),
                ("nl1W", [128, 3 * 128], f16),
                ("Ceff", [RB, 3 * 128], f16),
                ("e2w", [3, 3 * 128], f16),
                ("nl2W", [128, 3 * 128], f16),
                ("nl2bh", [128, L], f32),
                ("nl3W2", [128, 3 * 128], f16),
                ("dc0W", [128, 512], f16),
                ("dc1W", [128, 128], f16),
                ("dc2W", [128, 128], f16),
                ("dc3W", [128, 128], f16),
                ("dc4W", [128, 1], f16),
            ):
                t[nm] = pp.tile(shp, dt, name=f"t_{nm}")
                nc.sync.dma_start(t[nm][:], p[nm][:])
            h16_t = pp.tile([128, NPC], f16)
            nc.sync.dma_start(h16_t[:], p["h016"][:])
            h_t = pp.tile([128, NPC], f32)
            for c0 in range(0, NPC, 512):
                nc.scalar.activation(h_t[:, c0 : c0 + 512], h16_t[:, c0 : c0 + 512], AF.Copy)
            agg_sb = pp.tile([128, NPC], f32)
            agg16 = pp.tile([128, NPC], f16)

            def chunks(nsub):
                c = []
                s = 0
                while s < nsub:
                    n = min(ECHUNK // 128, nsub - s)
                    c.append((s, n))
                    s += n
                return c

            for l in range(L):
                wsl = slice(128 * l, 128 * (l + 1))
                # ---- hn = h @ nl1_W + b, node-major, publish + AllGather ----
                for g in range(NG):
                    gsl = slice(128 * g, 128 * (g + 1))
                    hnps = psN.tile([128, 128], f32, tag="nb")
                    nc.tensor.matmul(hnps[:], h16_t[:, gsl], t["nl1W"][:, wsl], start=True, stop=True)
                    hnnm = sp.tile([128, 128], f16)
                    nc.scalar.activation(hnnm[:], hnps[:], AF.Copy)
                    nc.sync.dma_start(ag_in[l][gsl, :], hnnm[:])
                nc.gpsimd.collective_compute(
                    "AllGather",
                    mybir.AluOpType.bypass,
                    replica_groups=[list(range(NCORES))],
                    ins=[ag_in[l][:]],
                    outs=[hn_all[l][:]],
                )

                # ---- edge passes ----
                open_ps = {}

                def edge_pass(subs, view, gidx_t, sub0_dstem, up_off, is_lo):
                    for s0, nsub in chunks(len(subs)):
                        ne = nsub * 128
                        hn_em = gp.tile([128, nsub, 128], f16)
                        nc.gpsimd.dma_gather(
                            hn_em[:], view, gidx_t[:, s0 * 8 : (s0 + nsub) * 8], ne, ne, 128
                        )
                        upt = up.tile([RB, ne], f16)
                        nc.sync.dma_start(
                            upt[:], p["UpT"][:, up_off + s0 * 128 : up_off + s0 * 128 + ne]
                        )
                        for j in range(nsub):
                            g, first, last = subs[s0 + j]
                            gsl = slice(128 * g, 128 * (g + 1))
                            if first:
                                aps = psA.tile([128, 128], f32)
                                open_ps[g] = aps
                                if is_lo:
                                    nc.tensor.matmul(
                                        aps[:], t["e2w"][:, wsl], t["cntT"][:, gsl],
                                        start=True, stop=False,
                                    )
                            aps = open_ps[g]
                            dps = psD.tile([128, 128], f32)
                            nc.tensor.matmul(
                                dps[:], upt[:, 128 * j : 128 * (j + 1)], t["Ceff"][:, wsl],
                                start=True, stop=True,
                            )
                            msg = sp.tile([128, 128], f16)
                            nc.vector.tensor_tensor(
                                out=msg[:], in0=dps[:], in1=hn_em[:, j, :], op=OP.mult
                            )
                            oh = sp.tile([128, 128], f16)
                            dc = sub0_dstem + s0 + j
                            nc.vector.tensor_tensor(
                                out=oh[:],
                                in0=t["dstem"][:, dc : dc + 1].to_broadcast([128, 128]),
                                in1=t["iota"][:],
                                op=OP.is_equal,
                            )
                            nc.tensor.matmul(
                                aps[:], msg[:], oh[:],
                                start=(first and not is_lo), stop=last,
                            )
                            if last:
                                if is_lo:
                                    nc.scalar.activation(agg_sb[:, gsl], aps[:], AF.Copy)
                                else:
                                    nc.vector.tensor_tensor(
                                        out=agg_sb[:, gsl], in0=aps[:], in1=agg_sb[:, gsl], op=OP.add
                                    )
                                del open_ps[g]

                edge_pass(subs_lo, hn_all[l][0:HALF, :], t["gidx_lo"], 0, 0, True)
                for g in layout["empty_lo"]:
                    gsl = slice(128 * g, 128 * (g + 1))
                    aps = psA.tile([128, 128], f32)
                    nc.tensor.matmul(
                        aps[:], t["e2w"][:, wsl], t["cntT"][:, gsl], start=True, stop=True
                    )
                    nc.scalar.activation(agg_sb[:, gsl], aps[:], AF.Copy)
                edge_pass(subs_hi, hn_all[l][HALF : NCORES * NPC, :], t["gidx_hi"], ESlo, Llo, False)

                # ---- node update ----
                for c0 in range(0, NPC, 512):
                    csl = slice(c0, c0 + 512)
                    nc.scalar.activation(agg16[:, csl], agg_sb[:, csl], AF.Copy)
                    g1ps = psN.tile([128, 512], f32, tag="nb")
                    nc.tensor.matmul(g1ps[:], t["nl2W"][:, wsl], agg16[:, csl], start=True, stop=True)
                    ex = npo.tile([128, 512], f32)
                    nc.scalar.activation(
                        ex[:], g1ps[:], AF.Exp, bias=t["nl2bh"][:, l : l + 1], scale=0.5
                    )
                    sph = npo.tile([128, 512], f16)
                    nc.scalar.activation(sph[:], ex[:], AF.Ln, bias=1.0)
                    g2ps = psN.tile([128, 512], f32, tag="nb")
                    nc.tensor.matmul(g2ps[:], t["nl3W2"][:, wsl], sph[:], start=True, stop=True)
                    nc.vector.tensor_tensor(
                        out=h_t[:, csl], in0=g2ps[:], in1=h_t[:, csl], op=OP.add
                    )
                    nc.scalar.activation(h16_t[:, csl], h_t[:, csl], AF.Copy)
                    if l < 2:
                        nc.sync.dma_start(snapd[l][:, csl], h16_t[:, csl])

            # ---- decoder ----
            for c0 in range(0, NPC, 512):
                csl = slice(c0, c0 + 512)
                rhs = []
                for srcd in (p["h016"], snapd[0], snapd[1]):
                    rt = npo.tile([128, 512], f16)
                    nc.sync.dma_start(rt[:], srcd[:, csl])
                    rhs.append(rt)
                yps = psN.tile([128, 512], f32, tag="nb")
                for i in range(3):
                    nc.tensor.matmul(
                        yps[:], t["dc0W"][:, 128 * i : 128 * (i + 1)], rhs[i][:],
                        start=(i == 0), stop=False,
                    )
                nc.tensor.matmul(yps[:], t["dc0W"][:, 384:512], h16_t[:, csl], start=False, stop=True)
                ycur = None
                for i, (wt, al) in enumerate(
                    (
                        ("dc0W", prelu_a[0]),
                        ("dc1W", prelu_a[1]),
                        ("dc2W", prelu_a[2]),
                        ("dc3W", prelu_a[3]),
                    )
                ):
                    if i > 0:
                        yps = psN.tile([128, 512], f32, tag="nb")
                        nc.tensor.matmul(yps[:], t[wt][:], ycur[:], start=True, stop=True)
                    ya = npo.tile([128, 512], f32)
                    nc.scalar.activation(ya[:], yps[:], AF.Copy)
                    ycur = npo.tile([128, 512], f16)
                    nc.vector.scalar_tensor_tensor(
                        ycur[:], in0=ya[:], scalar=al, in1=ya[:], op0=OP.mult, op1=OP.max
                    )
                ops_ = psN.tile([1, 512], f32, tag="nb")
                nc.tensor.matmul(ops_[:], t["dc4W"][:], ycur[:], start=True, stop=True)
                osb = npo.tile([1, 512], f32)
                nc.scalar.activation(osb[:], ops_[:], AF.Copy)
                nc.sync.dma_start(out[:, csl], osb[:])

    return nc


def _make_runner(nc, in_maps):
    # Persistent SPMD runner: jit once, upload inputs once, reuse across runs.
    import jax
    from jax.sharding import Mesh, PartitionSpec, NamedSharding
    from jax.experimental.shard_map import shard_map
    from concourse import mybir
    from concourse.bass2jax import (
        _bass_exec_p,
        install_neuronx_cc_hook,
        partition_id_tensor,
    )

    install_neuronx_cc_hook()
    n_cores = len(in_maps)
    partition_name = nc.partition_id_tensor.name if nc.partition_id_tensor else None
    in_names, out_names, out_avals = [], [], []
    for alloc in nc.m.functions[0].allocations:
        if not isinstance(alloc, mybir.MemoryLocationSet):
            continue
        name = alloc.memorylocations[0].name
        if alloc.kind == "ExternalInput":
            if name != partition_name:
                in_names.append(name)
        elif alloc.kind == "ExternalOutput":
            out_names.append(name)
            out_avals.append(
                jax.core.ShapedArray(tuple(alloc.tensor_shape), mybir.dt.np(alloc.dtype))
            )
    n_params = len(in_names)
    n_outs = len(out_names)
    all_names = tuple(in_names + out_names + ([partition_name] if partition_name else []))

    def _body(*args):
        operands = list(args)
        if partition_name is not None:
            operands.append(partition_id_tensor())
        return tuple(
            _bass_exec_p.bind(
                *operands,
                out_avals=tuple(out_avals),
                in_names=all_names,
                out_names=tuple(out_names),
                lowering_input_output_aliases=(),
                sim_require_finite=True,
                sim_require_nnan=True,
                nc=nc,
            )
        )

    devices = jax.devices()[:n_cores]
    mesh = Mesh(np.asarray(devices), ("core",))
    in_specs = (PartitionSpec("core"),) * (n_params + n_outs)
    out_specs = (PartitionSpec("core"),) * n_outs
    sharded = jax.jit(
        shard_map(_body, mesh=mesh, in_specs=in_specs, out_specs=out_specs, check_rep=False),
        keep_unused=True,
    )
    sh = NamedSharding(mesh, PartitionSpec("core"))
    concat_in = [
        np.concatenate([np.asarray(m[nm]) for m in in_maps], axis=0) for nm in in_names
    ]
    dev_in = [jax.device_put(a, sh) for a in concat_in]
    # "out" is fully written by the kernel (all NPC columns), so the seed
    # buffers are never read — keep them resident and undonated.
    dev_zeros = [
        jax.device_put(np.zeros((n_cores * o.shape[0], *o.shape[1:]), o.dtype), sh)
        for o in out_avals
    ]
    jax.block_until_ready(dev_in)
    jax.block_until_ready(dev_zeros)

    def run():
        outs = sharded(*dev_in, *dev_zeros)
        fetched = [np.asarray(a) for a in outs]
        return [
            {
                nm: fetched[i].reshape(n_cores, *out_avals[i].shape)[c]
                for i, nm in enumerate(out_names)
            }
            for c in range(n_cores)
        ]

    return run


TRACE = False
LAST_EXEC_NS = None
LAST_WALL_NS = None


def kernel(**inputs):
    global LAST_EXEC_NS, LAST_WALL_NS
    import time

    percore, wts, layout = _host_prep(inputs)

    nc = _build(layout)
    nc.compile()
    in_maps = [{**pc, **wts} for pc in percore]
    run = _make_runner(nc, in_maps)
    results = run()
    if TRACE:
        t0 = time.perf_counter()
        results = run()
        LAST_WALL_NS = int((time.perf_counter() - t0) * 1e9)
    outv = np.empty((N, 1), f32d)
    for k in range(NCORES):
        outv[NPN * k : NPN * (k + 1), 0] = results[k]["out"][0, :NPN]
    return outv

